# revision 38
# baseline (speedup 1.0000x reference)
"""Trainium2 Bass kernel for nn_Attn_45423574123081 (sparse_attention).

Computes, for inputs enc [B=32, L=1024, D=64], W [64, 64], b [64]:
    energy = enc @ W.T + b                       # [B, L, D]
    scores[t, b, j] = energy[b, j] . enc[b, t]   # [T=1024, B, L]
    scores[t, :, t] = 0
    out = softmax(scores, axis=-1)

Sharding: data-parallel over batch; 4 batches per core on 8 NeuronCores.

v2 redesign (70.1us -> ~59us +-1us env drift, all trace-validated;
steady state 38.1us gap-free, startup ~14.5us, tail ~3.3us + ~2.6us
fixed teardown + ~6.6us fixed preamble):
 * Interleaved t-layout (t = 8p + n): enc loads become 128 contiguous
   2KB descriptors instead of 1024x256B (the old batch-0 load took
   ~10.5us and gated the first chunk at 17.3us).  All batches cast
   f32->f16 in-flight on the SWDGE queue (goes live ~1us before the
   HWDGE rings); masks/stationaries ride the HWDGE rings in parallel.
 * Paired PE transposes [128,128]: even chunks' E^T on partitions 0-63,
   odd on 64-127.  K=64 score matmuls then alternate tile_position row
   groups (0,0)/(64,0) between consecutive chunks and run CONCURRENTLY
   in the PE array (46/72 overlap measured; old kernel's 78%-busy PE
   stalled ScalarE at batch seams).
 * W4x2 = [[W^T,W^T],[W^T,W^T]] and nb22 = [[-b,0],[0,-b]] are
   host-precomputed inputs (removes the on-device W/b prep chain from
   the startup critical path).
 * G^T via two concurrent row-group matmuls (stationary duplicates the
   output to both partition halves), interleave-merged f32->f16 by two
   strided copies (one on then-idle ScalarE for batch 0) so gt columns
   are in true t'-order; output DMA stays 2KB-contiguous per partition.
 * eb = -(E.b) per chunk-pair via one K=128 matmul against nb22.
 * Diagonal (softmax shift-invariance absorbs the +E.b bias): DVE
   predicated write of -c_t onto the stride-8 diagonal view ps[:, i::8].
 * Softmax: ScalarE exp f32 PSUM -> bf16 SBUF with fused accumulator
   row sums (READ_ACCUMULATOR is ~75% hidden behind the next ACTIVATE),
   DVE reciprocal + normalize, 512KB bf16 DMA per pair of t-blocks.
   Steady state is ridge-balanced: DVE per chunk (diag 296 + norm 481 +
   prep share ~335 = ~1112ns) == ScalarE ACTIVATE (1113ns), both ~100%
   busy for 38.3us; 'T'-mode (DVE tensor_tensor_reduce sums) does not
   fit in the DVE budget, so all chunks use accumulator sums.
 * Pair finishes deferred ~2 chunks so normalize work never bunches
   ahead of diagonal writes in the in-order DVE queue; last batch ships
   each 256KB t-block immediately, spread across sync/gpsimd/scalar
   queues, with the final chunk split across both HWDGE rings.
 * Fixed overheads: ~6.6us engine-queue preamble, ~2.6us teardown,
   ~1.3us ACT_TABLE_LOAD (hoisted via dummy exp at t=0).
"""

import numpy as np

_B, _L, _D, _T = 32, 1024, 64, 1024
_N_CORES = 8
_BPC = _B // _N_CORES  # batches per core

# Per-chunk row-sum mode: 'A' = ScalarE accumulator (+~290ns READ_ACC on
# ScalarE), 'T' = DVE tensor_tensor_reduce (~0.6us on DVE).
_MODES = ["A", "A", "A", "A", "A", "A", "A", "A"]

_compiled_nc = None


def _build():
    global _compiled_nc
    if _compiled_nc is not None:
        return _compiled_nc

    import concourse.bacc as bacc
    import concourse.mybir as mybir
    from concourse import tile

    dt = mybir.dt
    AF = mybir.ActivationFunctionType
    ALU = mybir.AluOpType

    nc = bacc.Bacc(
        "TRN2",
        target_bir_lowering=False,
        debug=False,
        enable_asserts=False,
        num_devices=_N_CORES,
    )
    enc_d = nc.dram_tensor("enc", [_BPC, _L, _D], dt.float32, kind="ExternalInput")
    # host-precomputed stationaries: [[W^T, W^T], [W^T, W^T]] and
    # [[-b, 0], [0, -b]] (removes the whole on-device W/b prep chain
    # from the startup critical path)
    w4_d = nc.dram_tensor("w4x2", [128, 128], dt.float16, kind="ExternalInput")
    nb_d = nc.dram_tensor("nb22", [128, 2], dt.float16, kind="ExternalInput")
    id16_d = nc.dram_tensor("ident16", [128, 128], dt.float16, kind="ExternalInput")
    id8_d = nc.dram_tensor("ident8", [128, 128], dt.int8, kind="ExternalInput")
    out_d = nc.dram_tensor("out", [_T, _BPC, _L], dt.bfloat16, kind="ExternalOutput")

    with tile.TileContext(nc) as tc:
        with (
            tc.tile_pool(name="const", bufs=1) as cpool,
            tc.tile_pool(name="encp", bufs=2) as encpool,
            tc.tile_pool(name="etp", bufs=2) as etpool,
            tc.tile_pool(name="gtp", bufs=2) as gtpool,
            tc.tile_pool(name="ebp", bufs=2) as ebpool,
            tc.tile_pool(name="expp", bufs=7) as exppool,
            tc.tile_pool(name="outp", bufs=3) as outpool,
            tc.tile_pool(name="sump", bufs=2) as sumpool,
            tc.tile_pool(name="scrp", bufs=2) as scrpool,
            tc.tile_pool(name="ps_s", bufs=3, space="PSUM") as ps_s_pool,
            tc.tile_pool(name="ps_m", bufs=2, space="PSUM") as ps_m_pool,
        ):
            # Dummy exp at t=0 hoists the ~2.7us ACT_TABLE_LOAD off the
            # first chunk's critical path.
            warm = cpool.tile([1, 2], dt.float32)
            nc.vector.memset(warm[:], 0.0)
            nc.scalar.activation(warm[:, 0:1], warm[:, 1:2], AF.Exp)

            # enc batch 0 goes f32 over the fast HWDGE sync queue (2KB
            # contiguous per partition) and is cast f32->f16 on the
            # still-idle DVE; batches 1-3 use the SWDGE casting loads
            # off the critical path.  Interleaved layout: partition p
            # holds rows t = 8p+0..8p+7.
            # enc batch 0 cast-loads f16 directly on the SWDGE queue
            # (gpsimd goes live ~1us before the HWDGE rings and the f16
            # load skips the DVE cast), split in two halves so batch-0
            # prep (transposes, G column-group A, eb) pipelines against
            # the second half's transfer; masks + stationaries go
            # through the sync HWDGE ring in parallel.
            enc16_b0 = encpool.tile([128, 8 * _D], dt.float16, tag="enc16")
            for h in range(2):
                nc.gpsimd.dma_start(
                    enc16_b0[:, h * 4 * _D : (h + 1) * 4 * _D].rearrange(
                        "p (n d) -> p n d", n=4
                    ),
                    enc_d[0].rearrange("(p n) d -> p n d", p=128)[
                        :, h * 4 : (h + 1) * 4
                    ],
                )
            ident16 = cpool.tile([128, 128], dt.float16)
            nc.sync.dma_start(ident16[:], id16_d[:])
            w4x2 = cpool.tile([128, 128], dt.float16)
            nc.sync.dma_start(w4x2[:], w4_d[:])
            nb22 = cpool.tile([128, 2], dt.float16)
            nc.sync.dma_start(nb22[:], nb_d[:])
            ident_i8 = cpool.tile([128, 128], dt.int8)
            nc.scalar.dma_start(ident_i8[:], id8_d[:])

            def prep_load(bb):
                """enc f32 DRAM -> f16 SBUF (cast on SWDGE), interleaved."""
                enc16 = encpool.tile([128, 8 * _D], dt.float16, tag="enc16")
                nc.gpsimd.dma_start(
                    enc16[:].rearrange("p (n d) -> p n d", n=8),
                    enc_d[bb].rearrange("(p n) d -> p n d", p=128),
                )
                return enc16

            def prep_tr(bb, enc16, eng=None):
                """4 paired transposes: et2 [128, 512] f16, pair q block
                at cols q*128..: rows 0-63 = E^T for chunk 2q (t=8p+2q),
                rows 64-127 = chunk 2q+1.  eng picks the PSUM->SBUF copy
                engine (ScalarE for batch 0, when it is still idle)."""
                ps_et = ps_m_pool.tile([128, 512], dt.float16, tag="ps_m")
                for q in range(4):
                    nc.tensor.transpose(
                        ps_et[:, q * 128 : (q + 1) * 128],
                        enc16[:, q * 128 : (q + 1) * 128],
                        ident16[:],
                    )
                et2 = etpool.tile([128, 512], dt.float16, tag="et2")
                if eng is None:
                    nc.vector.tensor_copy(et2[:], ps_et[:])
                else:
                    eng.copy(et2[:], ps_et[:])
                return et2

            def et_sl(et2, i):
                """lhsT slice [64, 128] for chunk i (row half i%2)."""
                q, h = i // 2, i % 2
                return et2[64 * h : 64 * h + 64, q * 128 : (q + 1) * 128]

            def prep_g_mm(bb, et2):
                """G MMs: two concurrent row-group MMs (even / odd
                chunks) into two PSUM banks, stacked (q,p) col order."""
                gt2 = gtpool.tile([128, _L], dt.float16, tag="gt2")
                ps_gs = []
                for h in range(2):
                    ps_g = ps_m_pool.tile([128, 512], dt.float32, tag="ps_m")
                    nc.tensor.matmul(
                        ps_g[:],
                        w4x2[64 * h : 64 * h + 64, :],
                        et2[64 * h : 64 * h + 64, :],
                        start=True,
                        stop=True,
                    )
                    ps_gs.append(ps_g)
                return gt2, ps_gs

            def prep_g_cast(gt2, ps_gs, h, eng=None):
                """Interleave-merge one parity into gt (t' = 8p+2q+h).
                Emitted one half per chunk slot so the 661ns casts never
                stack up between diag writes in the in-order DVE queue.
                src re-walked (q,p)->(p,q) to match the ascending-t'
                destination view."""
                gview = gt2[:].rearrange("p (pp h) -> p h pp", h=2)
                dst = gview[:, h : h + 1].squeeze(1)
                src = ps_gs[h][:].rearrange("p (q pp) -> p pp q", q=4)
                if eng is None:
                    nc.vector.tensor_copy(dst, src)
                else:
                    eng.copy(dst, src)

            def prep_eb_mm(bb, et2, qs):
                """ebn MMs for chunk pairs qs: col i = -c_t for chunk i
                rows (t = 8p + i).  One K=128 MM per chunk pair; split
                across two chunk slots so the PE queue never delays the
                next score matmul by more than ~330ns."""
                if qs[0] == 0:
                    prep_eb_mm.ps = ps_m_pool.tile([128, 8], dt.float32, tag="ps_m")
                for q in qs:
                    nc.tensor.matmul(
                        prep_eb_mm.ps[:, 2 * q : 2 * q + 2],
                        et2[:, q * 128 : (q + 1) * 128],
                        nb22[:],
                        start=True,
                        stop=True,
                    )

            def prep_eb_copy(bb):
                ebn = ebpool.tile([128, 8], dt.float32, tag="ebn")
                nc.vector.tensor_copy(ebn[:], prep_eb_mm.ps[:])
                return ebn

            def chunk(bb, i, et2, gt2, ebn, sums):
                """One t-block: 2 score MMs (N=512 halves; consecutive
                chunks alternate PE row groups so they overlap), diag
                write on the stride-8 view, exp, row sum."""
                mode = _MODES[i]
                h = i % 2
                ps = ps_s_pool.tile([128, _L], dt.float32, tag="ps_s")
                for s in range(2):
                    sl = slice(s * 512, (s + 1) * 512)
                    nc.tensor.matmul(
                        ps[:, sl],
                        et_sl(et2, i),
                        gt2[64 * h : 64 * h + 64, sl],
                        start=True,
                        stop=True,
                    )
                # diagonal of chunk i sits at (p, 8p + i): stride-8 view
                diag_view = ps[:].rearrange("p (pp e) -> p e pp", e=8)[
                    :, i : i + 1
                ].squeeze(1)
                nc.vector.copy_predicated(
                    diag_view,
                    ident_i8[:],
                    ebn[:, i : i + 1].to_broadcast([128, 128]),
                )
                exp_sb = exppool.tile([128, _L], dt.bfloat16, tag="exp")
                scol = sums[:, i : i + 1]
                if mode == "A":
                    nc.scalar.activation(exp_sb[:], ps[:], AF.Exp, accum_out=scol)
                else:
                    nc.scalar.activation(exp_sb[:], ps[:], AF.Exp)
                    scr = scrpool.tile([128, 512], dt.bfloat16, tag="scr")
                    nc.vector.tensor_tensor_reduce(
                        scr[:],
                        exp_sb[:, 0:512],
                        exp_sb[:, 512:1024],
                        1.0,
                        0.0,
                        ALU.add,
                        ALU.add,
                        accum_out=scol,
                    )
                return exp_sb

            out_r = out_d.rearrange("(p e) b j -> p e b j", e=8)

            def finish_pair(bb, q, exps, sums, recips):
                """Reciprocal for chunks 2q/2q+1, normalize, DMA out."""
                pr = slice(2 * q, 2 * q + 2)
                nc.vector.reciprocal(recips[:, pr], sums[:, pr])
                out16 = outpool.tile([128, 2 * _L], dt.bfloat16, tag="o16")
                for h in range(2):
                    i = 2 * q + h
                    nc.vector.tensor_scalar_mul(
                        out16[:, h * _L : (h + 1) * _L],
                        exps[i][:],
                        recips[:, i : i + 1],
                    )
                dst = out_r[:, 2 * q : 2 * q + 2, bb : bb + 1, :].squeeze(2)
                nc.sync.dma_start(dst, out16[:].rearrange("p (e j) -> p e j", e=2))

            def finish_chunk(bb, i, exp_sb, sums, recips):
                """Tail-latency variant for the last batch: ship each
                256KB t-block as soon as its sum lands, spread across
                DMA-capable queues so the drains overlap.  The final
                chunk is split in two halves across both HWDGE rings
                (normalize h0 -> ship h0 while h1 normalizes)."""
                nc.vector.reciprocal(recips[:, i : i + 1], sums[:, i : i + 1])
                out16 = outpool.tile([128, _L], dt.bfloat16, tag="o16s")
                dst = out_r[:, i : i + 1, bb : bb + 1, :].squeeze(2).squeeze(1)
                if i < 7:
                    nc.vector.tensor_scalar_mul(
                        out16[:], exp_sb[:], recips[:, i : i + 1]
                    )
                    eng = {4: nc.sync, 5: nc.gpsimd, 6: nc.sync}[i]
                    eng.dma_start(dst, out16[:])
                else:
                    for h, eng in ((0, nc.scalar), (1, nc.sync)):
                        sl = slice(h * 512, (h + 1) * 512)
                        nc.vector.tensor_scalar_mul(
                            out16[:, sl], exp_sb[:, sl], recips[:, i : i + 1]
                        )
                        eng.dma_start(dst[:, sl], out16[:, sl])

            # --- software-pipelined emission ---------------------------------
            enc = [None] * _BPC
            et = [None] * _BPC
            gt = [None] * _BPC
            eb = [None] * _BPC
            enc[0] = enc16_b0
            enc[1] = prep_load(1)

            # --- batch-0 prep, pipelined against the second enc half:
            # transposes, eb matmuls and the G column-group for pairs
            # 0-1 all run while pairs 2-3 are still in flight.  G output
            # goes to one ps_s-pool tile (ga = cols 0:512, gb = 512:1024
            # -- distinct banks, so the row-group MMs stay concurrent).
            ps_et0 = ps_m_pool.tile([128, 512], dt.float16, tag="ps_m")
            et2_0 = etpool.tile([128, 512], dt.float16, tag="et2")
            ps_eb0 = ps_m_pool.tile([128, 8], dt.float32, tag="ps_m")
            ebn0 = ebpool.tile([128, 8], dt.float32, tag="ebn")
            ps_g0 = ps_s_pool.tile([128, _L], dt.float32, tag="ps_s")
            gt0 = gtpool.tile([128, _L], dt.float16, tag="gt2")
            gview4 = gt0[:].rearrange("p (pp q h) -> p h pp q", q=4, h=2)
            for g in range(2):  # column-group g = pairs 2g, 2g+1
                for q in (2 * g, 2 * g + 1):
                    nc.tensor.transpose(
                        ps_et0[:, q * 128 : (q + 1) * 128],
                        enc16_b0[:, q * 128 : (q + 1) * 128],
                        ident16[:],
                    )
                    nc.vector.tensor_copy(
                        et2_0[:, q * 128 : (q + 1) * 128],
                        ps_et0[:, q * 128 : (q + 1) * 128],
                    )
                for q in (2 * g, 2 * g + 1):
                    nc.tensor.matmul(
                        ps_eb0[:, 2 * q : 2 * q + 2],
                        et2_0[:, q * 128 : (q + 1) * 128],
                        nb22[:],
                        start=True,
                        stop=True,
                    )
                nc.vector.tensor_copy(
                    ebn0[:, 4 * g : 4 * g + 4], ps_eb0[:, 4 * g : 4 * g + 4]
                )
                csl = slice(g * 256, (g + 1) * 256)
                for h in range(2):
                    nc.tensor.matmul(
                        ps_g0[:, h * 512 + g * 256 : h * 512 + (g + 1) * 256],
                        w4x2[64 * h : 64 * h + 64, :],
                        et2_0[64 * h : 64 * h + 64, csl],
                        start=True,
                        stop=True,
                    )
                for h in range(2):
                    dst = gview4[:, h : h + 1, :, 2 * g : 2 * g + 2].squeeze(1)
                    src = ps_g0[
                        :, h * 512 + g * 256 : h * 512 + (g + 1) * 256
                    ].rearrange("p (q pp) -> p pp q", q=2)
                    if h == 0:
                        nc.scalar.copy(dst, src)
                    else:
                        nc.vector.tensor_copy(dst, src)
            et[0], gt[0], eb[0] = et2_0, gt0, ebn0

            pending = []
            for bb in range(_BPC):
                sums = sumpool.tile([128, 8], dt.float32, tag="sums")
                recips = sumpool.tile([128, 8], dt.float32, tag="recips")
                exps = [None] * 8
                last = bb == _BPC - 1
                for i in range(8):
                    exps[i] = chunk(bb, i, et[bb], gt[bb], eb[bb], sums)
                    # deferred pair finish: emit ~2 chunks after the pair
                    # completes so the in-order DVE queue never makes
                    # ScalarE wait on bunched normalize work (1-deep on
                    # the last batch to drain promptly)
                    if len(pending) >= (1 if last else 2):
                        pending.pop(0)()
                    if last and i >= 4:
                        finish_chunk(bb, i, exps[i], sums, recips)
                    elif i % 2 == 1:
                        pending.append(
                            lambda bb=bb, q=i // 2, e=exps, s=sums, r=recips:
                                finish_pair(bb, q, e, s, r)
                        )
                    if bb + 1 < _BPC:
                        if i == 3:
                            et[bb + 1] = prep_tr(bb + 1, enc[bb + 1])
                        elif i == 4:
                            gt[bb + 1], ps_gs = prep_g_mm(bb + 1, et[bb + 1])
                            prep_g_cast(gt[bb + 1], ps_gs, 0)
                        elif i == 5:
                            prep_g_cast(gt[bb + 1], ps_gs, 1)
                            prep_eb_mm(bb + 1, et[bb + 1], (0, 1))
                        elif i == 6:
                            prep_eb_mm(bb + 1, et[bb + 1], (2, 3))
                            eb[bb + 1] = prep_eb_copy(bb + 1)
                    if bb + 2 < _BPC and i == 2:
                        enc[bb + 2] = prep_load(bb + 2)

    nc.compile()
    _compiled_nc = nc
    return nc


def _numpy_fallback(enc, W, b, tl):
    energy = np.einsum("bld,ed->ble", enc, W) + b
    scores = np.einsum("bjd,btd->tbj", energy, enc[:, :tl, :])
    t_idx = np.arange(tl)
    scores[t_idx, :, t_idx] = 0.0
    m = scores.max(axis=-1, keepdims=True)
    e = np.exp(scores - m)
    return (e / e.sum(axis=-1, keepdims=True)).astype(np.float32)


def _run(encoder_outputs, W, b, target_length=1024, **run_kwargs):
    enc = np.ascontiguousarray(np.asarray(encoder_outputs, dtype=np.float32))
    Wn = np.ascontiguousarray(np.asarray(W, dtype=np.float32))
    bn = np.ascontiguousarray(np.asarray(b, dtype=np.float32))
    tl = int(target_length)
    if enc.shape != (_B, _L, _D) or tl != _T:
        return _numpy_fallback(enc, Wn, bn, tl), None

    from concourse.bass_utils import run_bass_kernel_spmd

    nc = _build()
    id16 = np.eye(128, dtype=np.float16)
    id8 = np.eye(128, dtype=np.int8)
    wt16 = Wn.T.astype(np.float16)  # [d, e]
    w4x2 = np.block([[wt16, wt16], [wt16, wt16]])  # [128, 128]
    nb16 = (-bn).astype(np.float16)
    nb22 = np.zeros((128, 2), dtype=np.float16)
    nb22[0:64, 0] = nb16
    nb22[64:128, 1] = nb16
    in_maps = [
        {
            "enc": enc[i * _BPC : (i + 1) * _BPC],
            "w4x2": w4x2,
            "nb22": nb22,
            "ident16": id16,
            "ident8": id8,
        }
        for i in range(_N_CORES)
    ]
    res = run_bass_kernel_spmd(nc, in_maps, list(range(_N_CORES)), **run_kwargs)
    out = np.concatenate(
        [np.asarray(res.results[i]["out"]) for i in range(_N_CORES)], axis=1
    ).astype(np.float32)
    return out, res


def kernel(encoder_outputs, W, b, target_length=1024):
    out, _ = _run(encoder_outputs, W, b, target_length)
    return out


def kernel_profiled(encoder_outputs, W, b, target_length=1024):
    """Run with NTFF tracing; returns (output, BassKernelResults)."""
    return _run(encoder_outputs, W, b, target_length, trace=True)


# revision 39
# speedup vs baseline: 1.0311x; 1.0311x over previous
"""Trainium2 Bass kernel for nn_Attn_45423574123081 (sparse_attention).

Computes, for inputs enc [B=32, L=1024, D=64], W [64, 64], b [64]:
    energy = enc @ W.T + b                       # [B, L, D]
    scores[t, b, j] = energy[b, j] . enc[b, t]   # [T=1024, B, L]
    scores[t, :, t] = 0
    out = softmax(scores, axis=-1)

Sharding: data-parallel over batch; 4 batches per core on 8 NeuronCores.

v2 redesign (70.1us -> ~59us +-1us env drift, all trace-validated;
steady state 38.1us gap-free, startup ~14.5us, tail ~3.3us + ~2.6us
fixed teardown + ~6.6us fixed preamble):
 * Interleaved t-layout (t = 8p + n): enc loads become 128 contiguous
   2KB descriptors instead of 1024x256B (the old batch-0 load took
   ~10.5us and gated the first chunk at 17.3us).  All batches cast
   f32->f16 in-flight on the SWDGE queue (goes live ~1us before the
   HWDGE rings); masks/stationaries ride the HWDGE rings in parallel.
 * Paired PE transposes [128,128]: even chunks' E^T on partitions 0-63,
   odd on 64-127.  K=64 score matmuls then alternate tile_position row
   groups (0,0)/(64,0) between consecutive chunks and run CONCURRENTLY
   in the PE array (46/72 overlap measured; old kernel's 78%-busy PE
   stalled ScalarE at batch seams).
 * W4x2 = [[W^T,W^T],[W^T,W^T]] and nb22 = [[-b,0],[0,-b]] are
   host-precomputed inputs (removes the on-device W/b prep chain from
   the startup critical path).
 * G^T via two concurrent row-group matmuls (stationary duplicates the
   output to both partition halves), interleave-merged f32->f16 by two
   strided copies (one on then-idle ScalarE for batch 0) so gt columns
   are in true t'-order; output DMA stays 2KB-contiguous per partition.
 * eb = -(E.b) per chunk-pair via one K=128 matmul against nb22.
 * Diagonal (softmax shift-invariance absorbs the +E.b bias): DVE
   predicated write of -c_t onto the stride-8 diagonal view ps[:, i::8].
 * Softmax: ScalarE exp f32 PSUM -> bf16 SBUF with fused accumulator
   row sums (READ_ACCUMULATOR is ~75% hidden behind the next ACTIVATE),
   DVE reciprocal + normalize, 512KB bf16 DMA per pair of t-blocks.
   Steady state is ridge-balanced: DVE per chunk (diag 296 + norm 481 +
   prep share ~335 = ~1112ns) == ScalarE ACTIVATE (1113ns), both ~100%
   busy for 38.3us; 'T'-mode (DVE tensor_tensor_reduce sums) does not
   fit in the DVE budget, so all chunks use accumulator sums.
 * Pair finishes deferred ~2 chunks so normalize work never bunches
   ahead of diagonal writes in the in-order DVE queue; last batch ships
   each 256KB t-block immediately, spread across sync/gpsimd/scalar
   queues, with the final chunk split across both HWDGE rings.
 * Fixed overheads: ~6.6us engine-queue preamble, ~2.6us teardown,
   ~1.3us ACT_TABLE_LOAD (hoisted via dummy exp at t=0).
"""

import numpy as np

_B, _L, _D, _T = 32, 1024, 64, 1024
_N_CORES = 8
_BPC = _B // _N_CORES  # batches per core

# Per-chunk row-sum mode: 'A' = ScalarE accumulator (+~290ns READ_ACC on
# ScalarE), 'T' = DVE tensor_tensor_reduce (~0.6us on DVE).
_MODES = ["A", "A", "A", "A", "A", "A", "A", "A"]

_compiled_nc = None


def _build():
    global _compiled_nc
    if _compiled_nc is not None:
        return _compiled_nc

    import concourse.bacc as bacc
    import concourse.mybir as mybir
    from concourse import tile

    dt = mybir.dt
    AF = mybir.ActivationFunctionType
    ALU = mybir.AluOpType

    nc = bacc.Bacc(
        "TRN2",
        target_bir_lowering=False,
        debug=False,
        enable_asserts=False,
        num_devices=_N_CORES,
    )
    enc_d = nc.dram_tensor("enc", [_BPC, _L, _D], dt.float32, kind="ExternalInput")
    # host-precomputed stationaries: [[W^T, W^T], [W^T, W^T]] and
    # [[-b, 0], [0, -b]] (removes the whole on-device W/b prep chain
    # from the startup critical path)
    w4_d = nc.dram_tensor("w4x2", [128, 128], dt.float16, kind="ExternalInput")
    nb_d = nc.dram_tensor("nb22", [128, 2], dt.float16, kind="ExternalInput")
    id16_d = nc.dram_tensor("ident16", [128, 128], dt.float16, kind="ExternalInput")
    id8_d = nc.dram_tensor("ident8", [128, 128], dt.int8, kind="ExternalInput")
    out_d = nc.dram_tensor("out", [_T, _BPC, _L], dt.bfloat16, kind="ExternalOutput")

    with tile.TileContext(nc) as tc:
        with (
            tc.tile_pool(name="const", bufs=1) as cpool,
            tc.tile_pool(name="encp", bufs=2) as encpool,
            tc.tile_pool(name="etp", bufs=2) as etpool,
            tc.tile_pool(name="gtp", bufs=2) as gtpool,
            tc.tile_pool(name="ebp", bufs=2) as ebpool,
            tc.tile_pool(name="expp", bufs=7) as exppool,
            tc.tile_pool(name="outp", bufs=3) as outpool,
            tc.tile_pool(name="sump", bufs=2) as sumpool,
            tc.tile_pool(name="scrp", bufs=2) as scrpool,
            tc.tile_pool(name="ps_s", bufs=3, space="PSUM") as ps_s_pool,
            tc.tile_pool(name="ps_m", bufs=2, space="PSUM") as ps_m_pool,
        ):
            # Dummy exp at t=0 hoists the ~2.7us ACT_TABLE_LOAD off the
            # first chunk's critical path.
            warm = cpool.tile([1, 2], dt.float32)
            nc.vector.memset(warm[:], 0.0)
            nc.scalar.activation(warm[:, 0:1], warm[:, 1:2], AF.Exp)

            # enc batch 0 goes f32 over the fast HWDGE sync queue (2KB
            # contiguous per partition) and is cast f32->f16 on the
            # still-idle DVE; batches 1-3 use the SWDGE casting loads
            # off the critical path.  Interleaved layout: partition p
            # holds rows t = 8p+0..8p+7.
            # enc batch 0 cast-loads f16 directly on the SWDGE queue
            # (gpsimd goes live ~1us before the HWDGE rings and the f16
            # load skips the DVE cast); masks + stationaries go through
            # the sync HWDGE ring in parallel.
            enc16_b0 = encpool.tile([128, 8 * _D], dt.float16, tag="enc16")
            nc.gpsimd.dma_start(
                enc16_b0[:].rearrange("p (n d) -> p n d", n=8),
                enc_d[0].rearrange("(p n) d -> p n d", p=128),
            )
            ident16 = cpool.tile([128, 128], dt.float16)
            nc.sync.dma_start(ident16[:], id16_d[:])
            w4x2 = cpool.tile([128, 128], dt.float16)
            nc.sync.dma_start(w4x2[:], w4_d[:])
            nb22 = cpool.tile([128, 2], dt.float16)
            nc.sync.dma_start(nb22[:], nb_d[:])
            ident_i8 = cpool.tile([128, 128], dt.int8)
            nc.scalar.dma_start(ident_i8[:], id8_d[:])

            def prep_load(bb):
                """enc f32 DRAM -> f16 SBUF (cast on SWDGE), interleaved."""
                enc16 = encpool.tile([128, 8 * _D], dt.float16, tag="enc16")
                nc.gpsimd.dma_start(
                    enc16[:].rearrange("p (n d) -> p n d", n=8),
                    enc_d[bb].rearrange("(p n) d -> p n d", p=128),
                )
                return enc16

            def prep_tr(bb, enc16, eng=None):
                """4 paired transposes: et2 [128, 512] f16, pair q block
                at cols q*128..: rows 0-63 = E^T for chunk 2q (t=8p+2q),
                rows 64-127 = chunk 2q+1.  eng picks the PSUM->SBUF copy
                engine (ScalarE for batch 0, when it is still idle)."""
                ps_et = ps_m_pool.tile([128, 512], dt.float16, tag="ps_m")
                for q in range(4):
                    nc.tensor.transpose(
                        ps_et[:, q * 128 : (q + 1) * 128],
                        enc16[:, q * 128 : (q + 1) * 128],
                        ident16[:],
                    )
                et2 = etpool.tile([128, 512], dt.float16, tag="et2")
                if eng is None:
                    nc.vector.tensor_copy(et2[:], ps_et[:])
                else:
                    eng.copy(et2[:], ps_et[:])
                return et2

            def et_sl(et2, i):
                """lhsT slice [64, 128] for chunk i (row half i%2)."""
                q, h = i // 2, i % 2
                return et2[64 * h : 64 * h + 64, q * 128 : (q + 1) * 128]

            def prep_g_mm(bb, et2):
                """G MMs: two concurrent row-group MMs (even / odd
                chunks) into two PSUM banks, stacked (q,p) col order."""
                gt2 = gtpool.tile([128, _L], dt.float16, tag="gt2")
                ps_gs = []
                for h in range(2):
                    ps_g = ps_m_pool.tile([128, 512], dt.float32, tag="ps_m")
                    nc.tensor.matmul(
                        ps_g[:],
                        w4x2[64 * h : 64 * h + 64, :],
                        et2[64 * h : 64 * h + 64, :],
                        start=True,
                        stop=True,
                    )
                    ps_gs.append(ps_g)
                return gt2, ps_gs

            def prep_g_cast(gt2, ps_gs, h, eng=None):
                """Interleave-merge one parity into gt (t' = 8p+2q+h).
                Emitted one half per chunk slot so the 661ns casts never
                stack up between diag writes in the in-order DVE queue.
                src re-walked (q,p)->(p,q) to match the ascending-t'
                destination view."""
                gview = gt2[:].rearrange("p (pp h) -> p h pp", h=2)
                dst = gview[:, h : h + 1].squeeze(1)
                src = ps_gs[h][:].rearrange("p (q pp) -> p pp q", q=4)
                if eng is None:
                    nc.vector.tensor_copy(dst, src)
                else:
                    eng.copy(dst, src)

            def prep_eb_mm(bb, et2, qs):
                """ebn MMs for chunk pairs qs: col i = -c_t for chunk i
                rows (t = 8p + i).  One K=128 MM per chunk pair; split
                across two chunk slots so the PE queue never delays the
                next score matmul by more than ~330ns."""
                if qs[0] == 0:
                    prep_eb_mm.ps = ps_m_pool.tile([128, 8], dt.float32, tag="ps_m")
                for q in qs:
                    nc.tensor.matmul(
                        prep_eb_mm.ps[:, 2 * q : 2 * q + 2],
                        et2[:, q * 128 : (q + 1) * 128],
                        nb22[:],
                        start=True,
                        stop=True,
                    )

            def prep_eb_copy(bb):
                ebn = ebpool.tile([128, 8], dt.float32, tag="ebn")
                nc.vector.tensor_copy(ebn[:], prep_eb_mm.ps[:])
                return ebn

            def chunk(bb, i, et2, gt2, ebn, sums):
                """One t-block: 2 score MMs (N=512 halves; consecutive
                chunks alternate PE row groups so they overlap), diag
                write on the stride-8 view, exp, row sum."""
                mode = _MODES[i]
                h = i % 2
                ps = ps_s_pool.tile([128, _L], dt.float32, tag="ps_s")
                for s in range(2):
                    sl = slice(s * 512, (s + 1) * 512)
                    nc.tensor.matmul(
                        ps[:, sl],
                        et_sl(et2, i),
                        gt2[64 * h : 64 * h + 64, sl],
                        start=True,
                        stop=True,
                    )
                # diagonal of chunk i sits at (p, 8p + i): stride-8 view
                diag_view = ps[:].rearrange("p (pp e) -> p e pp", e=8)[
                    :, i : i + 1
                ].squeeze(1)
                nc.vector.copy_predicated(
                    diag_view,
                    ident_i8[:],
                    ebn[:, i : i + 1].to_broadcast([128, 128]),
                )
                exp_sb = exppool.tile([128, _L], dt.bfloat16, tag="exp")
                scol = sums[:, i : i + 1]
                if mode == "A":
                    nc.scalar.activation(exp_sb[:], ps[:], AF.Exp, accum_out=scol)
                else:
                    nc.scalar.activation(exp_sb[:], ps[:], AF.Exp)
                    scr = scrpool.tile([128, 512], dt.bfloat16, tag="scr")
                    nc.vector.tensor_tensor_reduce(
                        scr[:],
                        exp_sb[:, 0:512],
                        exp_sb[:, 512:1024],
                        1.0,
                        0.0,
                        ALU.add,
                        ALU.add,
                        accum_out=scol,
                    )
                return exp_sb

            out_r = out_d.rearrange("(p e) b j -> p e b j", e=8)

            def finish_pair(bb, q, exps, sums, recips):
                """Reciprocal for chunks 2q/2q+1, normalize, DMA out."""
                pr = slice(2 * q, 2 * q + 2)
                nc.vector.reciprocal(recips[:, pr], sums[:, pr])
                out16 = outpool.tile([128, 2 * _L], dt.bfloat16, tag="o16")
                for h in range(2):
                    i = 2 * q + h
                    nc.vector.tensor_scalar_mul(
                        out16[:, h * _L : (h + 1) * _L],
                        exps[i][:],
                        recips[:, i : i + 1],
                    )
                dst = out_r[:, 2 * q : 2 * q + 2, bb : bb + 1, :].squeeze(2)
                nc.sync.dma_start(dst, out16[:].rearrange("p (e j) -> p e j", e=2))

            def finish_chunk(bb, i, exp_sb, sums, recips):
                """Tail-latency variant for the last batch: ship each
                256KB t-block as soon as its sum lands, spread across
                DMA-capable queues so the drains overlap.  The final
                chunk is split in two halves across both HWDGE rings
                (normalize h0 -> ship h0 while h1 normalizes)."""
                nc.vector.reciprocal(recips[:, i : i + 1], sums[:, i : i + 1])
                out16 = outpool.tile([128, _L], dt.bfloat16, tag="o16s")
                dst = out_r[:, i : i + 1, bb : bb + 1, :].squeeze(2).squeeze(1)
                if i < 7:
                    nc.vector.tensor_scalar_mul(
                        out16[:], exp_sb[:], recips[:, i : i + 1]
                    )
                    eng = {4: nc.sync, 5: nc.gpsimd, 6: nc.sync}[i]
                    eng.dma_start(dst, out16[:])
                else:
                    for h, eng in ((0, nc.scalar), (1, nc.sync)):
                        sl = slice(h * 512, (h + 1) * 512)
                        nc.vector.tensor_scalar_mul(
                            out16[:, sl], exp_sb[:, sl], recips[:, i : i + 1]
                        )
                        eng.dma_start(dst[:, sl], out16[:, sl])

            # --- software-pipelined emission ---------------------------------
            enc = [None] * _BPC
            et = [None] * _BPC
            gt = [None] * _BPC
            eb = [None] * _BPC
            enc[0] = enc16_b0
            enc[1] = prep_load(1)  # first op on the SWDGE queue
            et[0] = prep_tr(0, enc[0])
            gt[0], ps_gs0 = prep_g_mm(0, et[0])
            prep_g_cast(gt[0], ps_gs0, 0, eng=nc.scalar)
            prep_g_cast(gt[0], ps_gs0, 1)
            prep_eb_mm(0, et[0], (0, 1, 2, 3))
            eb[0] = prep_eb_copy(0)

            pending = []
            for bb in range(_BPC):
                sums = sumpool.tile([128, 8], dt.float32, tag="sums")
                recips = sumpool.tile([128, 8], dt.float32, tag="recips")
                exps = [None] * 8
                last = bb == _BPC - 1
                for i in range(8):
                    exps[i] = chunk(bb, i, et[bb], gt[bb], eb[bb], sums)
                    # deferred pair finish: emit ~2 chunks after the pair
                    # completes so the in-order DVE queue never makes
                    # ScalarE wait on bunched normalize work (1-deep on
                    # the last batch to drain promptly)
                    if len(pending) >= (1 if last else 2):
                        pending.pop(0)()
                    if last and i >= 4:
                        finish_chunk(bb, i, exps[i], sums, recips)
                    elif i % 2 == 1:
                        pending.append(
                            lambda bb=bb, q=i // 2, e=exps, s=sums, r=recips:
                                finish_pair(bb, q, e, s, r)
                        )
                    if bb + 1 < _BPC:
                        if i == 3:
                            et[bb + 1] = prep_tr(bb + 1, enc[bb + 1])
                        elif i == 4:
                            gt[bb + 1], ps_gs = prep_g_mm(bb + 1, et[bb + 1])
                            prep_g_cast(gt[bb + 1], ps_gs, 0)
                        elif i == 5:
                            prep_g_cast(gt[bb + 1], ps_gs, 1)
                            prep_eb_mm(bb + 1, et[bb + 1], (0, 1))
                        elif i == 6:
                            prep_eb_mm(bb + 1, et[bb + 1], (2, 3))
                            eb[bb + 1] = prep_eb_copy(bb + 1)
                    if bb + 2 < _BPC and i == 2:
                        enc[bb + 2] = prep_load(bb + 2)

    nc.compile()
    _compiled_nc = nc
    return nc


def _numpy_fallback(enc, W, b, tl):
    energy = np.einsum("bld,ed->ble", enc, W) + b
    scores = np.einsum("bjd,btd->tbj", energy, enc[:, :tl, :])
    t_idx = np.arange(tl)
    scores[t_idx, :, t_idx] = 0.0
    m = scores.max(axis=-1, keepdims=True)
    e = np.exp(scores - m)
    return (e / e.sum(axis=-1, keepdims=True)).astype(np.float32)


def _run(encoder_outputs, W, b, target_length=1024, **run_kwargs):
    enc = np.ascontiguousarray(np.asarray(encoder_outputs, dtype=np.float32))
    Wn = np.ascontiguousarray(np.asarray(W, dtype=np.float32))
    bn = np.ascontiguousarray(np.asarray(b, dtype=np.float32))
    tl = int(target_length)
    if enc.shape != (_B, _L, _D) or tl != _T:
        return _numpy_fallback(enc, Wn, bn, tl), None

    from concourse.bass_utils import run_bass_kernel_spmd

    nc = _build()
    id16 = np.eye(128, dtype=np.float16)
    id8 = np.eye(128, dtype=np.int8)
    wt16 = Wn.T.astype(np.float16)  # [d, e]
    w4x2 = np.block([[wt16, wt16], [wt16, wt16]])  # [128, 128]
    nb16 = (-bn).astype(np.float16)
    nb22 = np.zeros((128, 2), dtype=np.float16)
    nb22[0:64, 0] = nb16
    nb22[64:128, 1] = nb16
    in_maps = [
        {
            "enc": enc[i * _BPC : (i + 1) * _BPC],
            "w4x2": w4x2,
            "nb22": nb22,
            "ident16": id16,
            "ident8": id8,
        }
        for i in range(_N_CORES)
    ]
    res = run_bass_kernel_spmd(nc, in_maps, list(range(_N_CORES)), **run_kwargs)
    out = np.concatenate(
        [np.asarray(res.results[i]["out"]) for i in range(_N_CORES)], axis=1
    ).astype(np.float32)
    return out, res


def kernel(encoder_outputs, W, b, target_length=1024):
    out, _ = _run(encoder_outputs, W, b, target_length)
    return out


def kernel_profiled(encoder_outputs, W, b, target_length=1024):
    """Run with NTFF tracing; returns (output, BassKernelResults)."""
    return _run(encoder_outputs, W, b, target_length, trace=True)
